# revision 21
# baseline (speedup 1.0000x reference)
"""MultiHeadAttention (single-query cross-attention) Bass kernel for 8x TRN2.

Problem: B=32, S=4096, E=1024, H=16, D=64 (qk head dim), NV=64 (v head dim).
  q = seq1 @ Wq + bq                         [B,1,H*D]
  k = seq2 @ Wk + bk                         [B,S,H*D]
  v = seq2 @ Wv + bv                         [B,S,E]
  score = (q . k)/sqrt(D) per head, masked; attn = softmax(score)
  out = attn @ v                             [B,1,E]

Design (v2):
- Algebraic rewrite (query length 1): score[b,h,s] = seq2[b,s,:] . qk[b,h,:]
  with qk[b,h,:] = Wk[:,hD:hD+D] @ q[b,h,:]; out = ((attn @ seq2) @ Wv)/Z + bv.
  Drops the O(B*S*E*E) k/v projections entirely.
- Mask-aware gather: masked positions (mask==0) contribute exactly 0 to the
  softmax (exp(-1e9) == 0 in fp32), so the device only gathers UNMASKED rows
  of seq2 via indirect DMA (indices precomputed on host from the mask input).
  This halves HBM traffic and compute. Padding rows (to a multiple of 128) are
  killed with an additive -30000 bias fused into the exp() activation.
- bf16 on-chip: seq2 rows are cast to bf16 (1 cyc/col matmuls + cheap
  transposes), accumulation stays fp32 in PSUM. bk is dropped (uniform score
  shift cancels in softmax). Softmax runs without max subtraction (scores are
  ~N(0,1), exp is safe in fp32).
- s-major scores: per chunk, score^T[s,h] accumulates over 8 e-blocks with the
  transposed chunk block as the stationary operand, so exp+mask lands directly
  in the [s,h] layout the ctx matmul needs - no attention-weight transpose.
- Sharding: data-parallel over batch, 4 batches per core.
"""

import math
import os
import sys
import time

import numpy as np
import ml_dtypes

sys.path.insert(0, "/opt/trn_rl_repo")

import concourse.bacc as bacc
import concourse.bass as bass
import concourse.mybir as mybir
import concourse.tile as tile
from concourse.bass_utils import run_bass_kernel_spmd
import concourse.bass_utils as _bu

if os.environ.get("KLDW", "0") == "1" and not getattr(_bu, "_ldw_patched", False):
    _orig_run_command = _bu.run_command

    def _run_command_ldw(cmd, *a, **kw):
        if isinstance(cmd, list):
            cmd = ["--enable-ldw-opt=true" if c == "--enable-ldw-opt=false" else c
                   for c in cmd]
        return _orig_run_command(cmd, *a, **kw)

    _bu.run_command = _run_command_ldw
    _bu._ldw_patched = True

N_CORES = 8
B, S, E = 32, 4096, 1024
H, D = 16, 64
B_LOC = B // N_CORES           # 4 batches per core
CH = 128                       # gathered rows per chunk (= SBUF partitions)
NE = E // 128                  # 8 e-blocks

F32 = mybir.dt.float32
F32R = mybir.dt.float32r
BF16 = mybir.dt.bfloat16
AF = mybir.ActivationFunctionType
NEG_BIAS = -30000.0            # additive pre-exp bias for padding rows


def build_nc(nch):
    """nch: chunks of 128 gathered rows per batch."""
    nc = bacc.Bacc("TRN2", target_bir_lowering=False, debug=False, num_devices=1)

    seq2 = nc.dram_tensor("seq2", [B_LOC * S, E], BF16, kind="ExternalInput").ap()
    s1t = nc.dram_tensor("s1t", [E, B_LOC], BF16, kind="ExternalInput").ap()
    idx = nc.dram_tensor("idx", [B_LOC * CH, nch], mybir.dt.int32, kind="ExternalInput").ap()
    bias = nc.dram_tensor("bias", [B_LOC * CH, nch], F32, kind="ExternalInput").ap()
    wq = nc.dram_tensor("wq", [E, E], BF16, kind="ExternalInput").ap()
    wkt = nc.dram_tensor("wkt", [E, E], BF16, kind="ExternalInput").ap()
    wv = nc.dram_tensor("wv", [E, E], BF16, kind="ExternalInput").ap()
    bq4 = nc.dram_tensor("bq4", [B_LOC, E], F32, kind="ExternalInput").ap()
    bv4 = nc.dram_tensor("bv4", [B_LOC, E], F32, kind="ExternalInput").ap()
    identb = nc.dram_tensor("identb", [128, 128], BF16, kind="ExternalInput").ap()
    onesd = nc.dram_tensor("onesd", [128, 1], BF16, kind="ExternalInput").ap()
    out = nc.dram_tensor("out", [B_LOC, E], F32, kind="ExternalOutput").ap()

    lin = os.environ.get("KLIN", "0") == "1"
    with tile.TileContext(nc, linearize=lin) as tc:
        _body(tc, nch, seq2, s1t, idx, bias, wq, wkt, wv, bq4, bv4, identb, onesd, out)
    nc.compile()
    return nc


def _body(tc, nch, seq2, s1t, idx, bias, wq, wkt, wv, bq4, bv4,
          identb, onesd, out):
    nc = tc.nc
    from contextlib import ExitStack

    with ExitStack() as stk:
        # ---- SBUF pools ------------------------------------------------
        consts = stk.enter_context(tc.tile_pool(name="consts", bufs=1))
        wpool = stk.enter_context(tc.tile_pool(name="wpool", bufs=1))
        chp = stk.enter_context(tc.tile_pool(name="chp", bufs=int(os.environ.get("KCHB", 24))))
        ctp = stk.enter_context(tc.tile_pool(name="ctp", bufs=int(os.environ.get("KCTB", 16))))
        wtp = stk.enter_context(tc.tile_pool(name="wtp", bufs=3))
        outp = stk.enter_context(tc.tile_pool(name="outp", bufs=1))

        # ---- constants -------------------------------------------------
        idx_sb = []
        bias_sb = []
        for b in range(B_LOC):
            t = consts.tile([CH, nch], mybir.dt.int32, tag=f"idx{b}", name=f"idx{b}")
            nc.gpsimd.dma_start(t[:], idx[b * CH:(b + 1) * CH, :])
            idx_sb.append(t)
            t = consts.tile([CH, nch], F32, tag=f"bias{b}", name=f"bias{b}")
            nc.sync.dma_start(t[:], bias[b * CH:(b + 1) * CH, :])
            bias_sb.append(t)
        identb_sb = consts.tile([128, 128], BF16, tag="identb", name="identb")
        nc.sync.dma_start(identb_sb[:], identb[:])
        ones_sb = consts.tile([128, 1], BF16, tag="ones", name="ones")
        nc.sync.dma_start(ones_sb[:], onesd[:])
        s1t_sb = []
        for j in range(NE):
            t = consts.tile([128, B_LOC], BF16, tag=f"s1t{j}", name=f"s1t{j}")
            nc.sync.dma_start(t[:], s1t[j * 128:(j + 1) * 128, :])
            s1t_sb.append(t)
        bq4_sb = consts.tile([B_LOC, E], F32, tag="bq4", name="bq4")
        nc.sync.dma_start(bq4_sb[:], bq4[:])
        bv4_sb = consts.tile([B_LOC, E], F32, tag="bv4", name="bv4")
        nc.sync.dma_start(bv4_sb[:], bv4[:])

        wq_sb, wkt_sb, wv_sb = [], [], []
        for j in range(NE):
            t = wpool.tile([128, E], BF16, tag=f"wq{j}", name=f"wq{j}")
            nc.sync.dma_start(t[:], wq[j * 128:(j + 1) * 128, :])
            wq_sb.append(t)
        for j in range(NE):
            t = wpool.tile([128, E], BF16, tag=f"wkt{j}", name=f"wkt{j}")
            nc.sync.dma_start(t[:], wkt[j * 128:(j + 1) * 128, :])
            wkt_sb.append(t)

        # ---- PSUM pools (8 banks total) --------------------------------
        tpp = stk.enter_context(tc.tile_pool(name="tpp", bufs=int(os.environ.get("KTPB", 3)), space="PSUM"))
        spp = stk.enter_context(tc.tile_pool(name="spp", bufs=2, space="PSUM"))
        ctxp = stk.enter_context(tc.tile_pool(name="ctxp", bufs=1, space="PSUM"))
        zp = stk.enter_context(tc.tile_pool(name="zp", bufs=1, space="PSUM"))

        z_ps = zp.tile([H, 512], F32, tag="z", name="z")

        def filler(n=3):
            # Dependency-free PE busywork into unused Z-bank columns: keeps
            # the HAM activity window hot so real matmuls stay at 2.4 GHz.
            # Safe: per-element has_written bits; cols 0:4 (real Z) untouched.
            for _ in range(n):
                nc.tensor.matmul(z_ps[:, 8:136], identb_sb[:, 0:H],
                                 identb_sb[:],
                                 start=False, stop=False,
                                 skip_group_check=True)

        # ---- chunk load helper (gather + transpose + evacuate) ---------
        cast_dma = os.environ.get("KCAST", "dma") == "dma"

        def load_chunk(b, c, ci):
            if cast_dma:
                ch = chp.tile([CH, E], BF16, tag="ch", name="ch")
                nc.gpsimd.indirect_dma_start(
                    out=ch[:], out_offset=None, in_=seq2[:],
                    in_offset=bass.IndirectOffsetOnAxis(
                        ap=idx_sb[b][:, c:c + 1], axis=0))
            else:
                chf = chp.tile([CH, E], F32, tag="chf", name="chf")
                nc.gpsimd.indirect_dma_start(
                    out=chf[:], out_offset=None, in_=seq2[:],
                    in_offset=bass.IndirectOffsetOnAxis(
                        ap=idx_sb[b][:, c:c + 1], axis=0))
                ch = chp.tile([CH, E], BF16, tag="ch", name="ch")
                if ci % 2 == 0:
                    nc.vector.tensor_copy(ch[:], chf[:])
                else:
                    nc.scalar.copy(ch[:], chf[:])
            # transpose the chunk: tp[:, j*128+s] = ch[s, j*128+e]
            tp = tpp.tile([CH, E], BF16, tag="tp", name="tp")
            for j in range(NE):
                nc.tensor.transpose(tp[:, j * 128:(j + 1) * 128],
                                    ch[:, j * 128:(j + 1) * 128],
                                    identb_sb[:])
            ct = ctp.tile([CH, E], BF16, tag="ct", name="ct")
            nc.vector.tensor_copy(ct[:], tp[:])
            return ch, ct

        # Warmup prepass: the PE executes in program order, so emit the first
        # chunks' transposes before the prologue matmuls - they only need the
        # gather + identity, which land long before the 6 MB of weights.
        warm = min(int(os.environ.get("KWARM", 4)), nch)
        pre = {}
        for c in range(warm):
            pre[(0, c)] = load_chunk(0, c, c)
        for j in range(NE):
            t = wpool.tile([128, E], BF16, tag=f"wv{j}", name=f"wv{j}")
            nc.sync.dma_start(t[:], wv[j * 128:(j + 1) * 128, :])
            wv_sb.append(t)

        # ================= prologue: q and qk =========================
        # q = seq1 @ Wq + bq   -> [B_LOC, E]
        q_ps = ctxp.tile([B_LOC, E], F32, tag="ctx", name="q_ps")
        for j in range(NE):
            for hf in range(2):
                nc.tensor.matmul(q_ps[:, hf * 512:(hf + 1) * 512], s1t_sb[j][:],
                                 wq_sb[j][:, hf * 512:(hf + 1) * 512],
                                 start=(j == 0), stop=(j == NE - 1),
                                 skip_group_check=True)
        q_sb = consts.tile([B_LOC, E], BF16, tag="q", name="q")
        nc.vector.tensor_add(q_sb[:], q_ps[:], bq4_sb[:])
        # qT blocks [128(hd), B_LOC] in bf16
        qt_sb = []
        for j in range(NE):
            ps = spp.tile([128, 64], BF16, tag="sp", name="qt_ps")
            nc.tensor.transpose(ps[:, 0:B_LOC], q_sb[:, j * 128:(j + 1) * 128],
                                identb_sb[0:B_LOC, 0:B_LOC])
            t = consts.tile([128, B_LOC], BF16, tag=f"qt{j}", name=f"qt{j}")
            nc.vector.tensor_copy(t[:], ps[:, 0:B_LOC])
            qt_sb.append(t)
        # qk[e, b*H+h] = sum_d WkT[h*64+d, e] * qT[h*64+d, b]
        # Block-diagonal trick: per hd-block, build qtbd [128, (b, par)] with
        # the other head-half zeroed, so one matmul against the FULL 128-row
        # WkT block computes both heads of the block at once (64 MMs not 128).
        qtbd = []
        for hj in range(NE):
            t = consts.tile([128, 2 * B_LOC], BF16, tag=f"qtbd{hj}",
                            name=f"qtbd{hj}")
            nc.vector.memset(t[:], 0.0)
            tr = t.rearrange("p (b q) -> p b q", q=2)
            nc.vector.tensor_copy(tr[0:64, :, 0:1], qt_sb[hj][0:64, :, None])
            nc.vector.tensor_copy(tr[64:128, :, 1:2], qt_sb[hj][64:128, :, None])
            qtbd.append(t)
        qk_sb = []
        for j in range(NE):
            ps = spp.tile([128, 64], F32, tag="sp", name="qk_ps")
            psr = ps.rearrange("p (b h) -> p b h", h=H)
            for hj in range(NE):
                nc.tensor.matmul(psr[:, :, 2 * hj:2 * hj + 2],
                                 wkt_sb[hj][:, j * 128:(j + 1) * 128],
                                 qtbd[hj][:],
                                 start=True, stop=True,
                                 skip_group_check=True)
            t = consts.tile([128, 64], BF16, tag=f"qk{j}", name=f"qk{j}")
            nc.scalar.copy(t[:], ps[:])
            qk_sb.append(t)

        if os.environ.get("KPART") == "qk":
            dbg = outp.tile([B_LOC, E], F32, tag="osb", name="osb")
            nc.vector.tensor_copy(dbg[:, 0:64], qk_sb[0][0:B_LOC, :])
            nc.sync.dma_start(out[:], dbg[:])
            return

        # ================= main loop ==================================
        zr_sb = consts.tile([H, B_LOC], F32, tag="zr", name="zr")
        ctxn = [consts.tile([H, E], BF16, tag=f"ctxn{b}", name=f"ctxn{b}")
                for b in range(B_LOC)]
        cxt_sb = [consts.tile([128, B_LOC * H], BF16, tag=f"cxt{j}", name=f"cxt{j}")
                  for j in range(NE)]
        n_b = int(os.environ.get("KNB", B_LOC))
        for b in range(n_b):
            ctx_ps = ctxp.tile([H, E], F32, tag="ctx", name="ctx")
            for c in range(nch):
                ci = b * nch + c
                if (b, c) in pre:
                    ch, ct = pre.pop((b, c))
                else:
                    ch, ct = load_chunk(b, c, ci)
                # score^T[s, h] accumulated over e-blocks
                sp = spp.tile([CH, H], F32, tag="sp", name="sp")
                for j in range(NE):
                    nc.tensor.matmul(sp[:], ct[:, j * 128:(j + 1) * 128],
                                     qk_sb[j][:, b * H:(b + 1) * H],
                                     start=(j == 0), stop=(j == NE - 1),
                                     skip_group_check=True)
                # attn weights: exp(score/8 + pad_bias); pad rows -> exactly 0
                wt = wtp.tile([CH, H], BF16, tag="wt", name="wt")
                nc.scalar.activation(wt[:], sp[:], AF.Exp,
                                     bias=bias_sb[b][:, c:c + 1],
                                     scale=1.0 / (D ** 0.5))
                # ctx[h, e] += wt^T @ ch ; Z[h, b] += wt^T @ ones
                last = (c == nch - 1)
                for hf in range(2):
                    nc.tensor.matmul(ctx_ps[:, hf * 512:(hf + 1) * 512], wt[:],
                                     ch[:, hf * 512:(hf + 1) * 512],
                                     start=(c == 0), stop=last,
                                     skip_group_check=True)
                nc.tensor.matmul(z_ps[:, b:b + 1], wt[:], ones_sb[:],
                                 start=(b == 0 and c == 0),
                                 stop=(b == n_b - 1 and c == nch - 1),
                                 skip_group_check=True)
            # normalize: ctxn[b] = ctx / Z   (bf16 out)
            nc.vector.reciprocal(zr_sb[:, b:b + 1], z_ps[:, b:b + 1])
            filler(14)
            nc.vector.tensor_scalar_mul(ctxn[b][:], ctx_ps[:], zr_sb[:, b:b + 1])
            # transpose ctxn into cxt blocks for the output projection
            for j in range(NE):
                ps = spp.tile([128, H], BF16, tag="sp", name="cxt_ps")
                nc.tensor.transpose(ps[:], ctxn[b][:, j * 128:(j + 1) * 128],
                                    identb_sb[0:H, 0:H])
                nc.scalar.copy(cxt_sb[j][:, b * H:(b + 1) * H], ps[:])

        if os.environ.get("KPART") == "ctx":
            dbg = outp.tile([B_LOC, E], F32, tag="osb", name="osb")
            nc.vector.tensor_copy(dbg[:], ctxn[0][0:B_LOC, :])
            nc.sync.dma_start(out[:], dbg[:])
            return

        # ================= finale: out = (ctx/Z) @ Wv + bv =============
        filler(20)
        out_sb = outp.tile([B_LOC, E], F32, tag="osb", name="osb")
        for h in range(H):
            op = spp.tile([B_LOC, 64], F32, tag="sp", name="op")
            for j in range(NE):
                lhs = cxt_sb[j].rearrange("p (b h) -> p h b", h=H)
                nc.tensor.matmul(op[:], lhs[:, h:h + 1, :],
                                 wv_sb[j][:, h * 64:(h + 1) * 64],
                                 start=(j == 0), stop=(j == NE - 1),
                                 skip_group_check=True)
            nc.vector.tensor_add(out_sb[:, h * 64:(h + 1) * 64], op[:],
                                 bv4_sb[:, h * 64:(h + 1) * 64])
        nc.sync.dma_start(out[:], out_sb[:])


# --------------------------------------------------------------------------
_NC_CACHE = {}

# test-harness knobs (the grading harness never touches these)
TRACE = False
TRACE_DIR = None
LAST_RESULTS = None


def _get_nc(nch):
    if nch not in _NC_CACHE:
        _NC_CACHE[nch] = build_nc(nch)
    return _NC_CACHE[nch]


def make_in_maps(inputs):
    """Host-side staging: mask -> gather indices + pad bias, weight reformat."""
    seq1 = np.asarray(inputs["seq1"], dtype=np.float32)   # [B,1,E]
    seq2 = np.asarray(inputs["seq2"], dtype=np.float32)   # [B,S,E]
    mask = np.asarray(inputs["mask"])                     # [B,1,1,S] int32
    Wq = np.asarray(inputs["Wq"], dtype=np.float32)
    # bk dropped: uniform per-row score shift, cancels exactly in softmax.
    Wk = np.asarray(inputs["Wk"], dtype=np.float32)
    Wv = np.asarray(inputs["Wv"], dtype=np.float32)
    bq = np.asarray(inputs["bq"], dtype=np.float32)
    bv = np.asarray(inputs["bv"], dtype=np.float32)

    mf = mask.reshape(B, S) != 0
    counts = mf.sum(1)
    nch = max(1, math.ceil(int(counts.max()) / CH))
    sp = nch * CH
    idx = np.zeros((B, sp), np.int32)
    bias = np.full((B, sp), NEG_BIAS, np.float32)
    for b in range(B):
        rows = np.nonzero(mf[b])[0]
        n = len(rows)
        bloc = b % B_LOC
        idx[b, :n] = bloc * S + rows
        idx[b, n:] = bloc * S
        bias[b, :n] = 0.0

    identb = np.eye(128, dtype=ml_dtypes.bfloat16)
    onesd = np.ones((128, 1), dtype=ml_dtypes.bfloat16)
    bq4 = np.tile(bq[None, :], (B_LOC, 1)).astype(np.float32)
    bv4 = np.tile(bv[None, :], (B_LOC, 1)).astype(np.float32)
    wkt = np.ascontiguousarray(Wk.T).astype(ml_dtypes.bfloat16)
    wvb = Wv.astype(ml_dtypes.bfloat16)

    in_maps = []
    for core in range(N_CORES):
        b0 = core * B_LOC
        # [B_LOC*CH, nch]: [b*128+p, c] = value for chunk c, partition p
        idxc = idx[b0:b0 + B_LOC].reshape(B_LOC, nch, CH).transpose(0, 2, 1)
        biasc = bias[b0:b0 + B_LOC].reshape(B_LOC, nch, CH).transpose(0, 2, 1)
        in_maps.append({
            "seq2": np.ascontiguousarray(seq2[b0:b0 + B_LOC].reshape(B_LOC * S, E)).astype(ml_dtypes.bfloat16),
            "s1t": np.ascontiguousarray(seq1[b0:b0 + B_LOC, 0, :].T).astype(ml_dtypes.bfloat16),
            "idx": np.ascontiguousarray(idxc.reshape(B_LOC * CH, nch)),
            "bias": np.ascontiguousarray(biasc.reshape(B_LOC * CH, nch)),
            "wq": Wq.astype(ml_dtypes.bfloat16), "wkt": wkt, "wv": wvb,
            "bq4": bq4, "bv4": bv4, "identb": identb, "onesd": onesd,
        })
    return nch, in_maps


def kernel(**inputs):
    nch, in_maps = make_in_maps(inputs)
    nc = _get_nc(nch)

    global LAST_RESULTS
    kwargs = {}
    if TRACE:
        kwargs = {"trace": True, "tmpdir": TRACE_DIR}
    # Retry: a previously-faulted NeuronCore can be left wedged
    # (NRT_EXEC_UNIT_UNRECOVERABLE) and recovers after reset/re-init.
    last_exc = None
    for attempt in range(4):
        try:
            res = run_bass_kernel_spmd(nc, in_maps, list(range(N_CORES)), **kwargs)
            break
        except Exception as e:  # noqa: BLE001
            last_exc = e
            time.sleep(10 * (attempt + 1))
    else:
        raise last_exc
    LAST_RESULTS = res
    out = np.concatenate([res.results[c]["out"] for c in range(N_CORES)], axis=0)
    return out.reshape(B, 1, E)


if __name__ == "__main__":
    t0 = time.time()
    nc = build_nc(17)
    print(f"build+compile(py): {time.time() - t0:.1f}s")


# revision 22
# speedup vs baseline: 1.0151x; 1.0151x over previous
"""MultiHeadAttention (single-query cross-attention) Bass kernel for 8x TRN2.

Problem: B=32, S=4096, E=1024, H=16, D=64 (qk head dim), NV=64 (v head dim).
  q = seq1 @ Wq + bq                         [B,1,H*D]
  k = seq2 @ Wk + bk                         [B,S,H*D]
  v = seq2 @ Wv + bv                         [B,S,E]
  score = (q . k)/sqrt(D) per head, masked; attn = softmax(score)
  out = attn @ v                             [B,1,E]

Design (v2):
- Algebraic rewrite (query length 1): score[b,h,s] = seq2[b,s,:] . qk[b,h,:]
  with qk[b,h,:] = Wk[:,hD:hD+D] @ q[b,h,:]; out = ((attn @ seq2) @ Wv)/Z + bv.
  Drops the O(B*S*E*E) k/v projections entirely.
- Mask-aware gather: masked positions (mask==0) contribute exactly 0 to the
  softmax (exp(-1e9) == 0 in fp32), so the device only gathers UNMASKED rows
  of seq2 via indirect DMA (indices precomputed on host from the mask input).
  This halves HBM traffic and compute. Padding rows (to a multiple of 128) are
  killed with an additive -30000 bias fused into the exp() activation.
- bf16 on-chip: seq2 rows are cast to bf16 (1 cyc/col matmuls + cheap
  transposes), accumulation stays fp32 in PSUM. bk is dropped (uniform score
  shift cancels in softmax). Softmax runs without max subtraction (scores are
  ~N(0,1), exp is safe in fp32).
- s-major scores: per chunk, score^T[s,h] accumulates over 8 e-blocks with the
  transposed chunk block as the stationary operand, so exp+mask lands directly
  in the [s,h] layout the ctx matmul needs - no attention-weight transpose.
- Sharding: data-parallel over batch, 4 batches per core.
"""

import math
import os
import sys
import time

import numpy as np
import ml_dtypes

sys.path.insert(0, "/opt/trn_rl_repo")

import concourse.bacc as bacc
import concourse.bass as bass
import concourse.mybir as mybir
import concourse.tile as tile
from concourse.bass_utils import run_bass_kernel_spmd
import concourse.bass_utils as _bu

if os.environ.get("KLDW", "0") == "1" and not getattr(_bu, "_ldw_patched", False):
    _orig_run_command = _bu.run_command

    def _run_command_ldw(cmd, *a, **kw):
        if isinstance(cmd, list):
            cmd = ["--enable-ldw-opt=true" if c == "--enable-ldw-opt=false" else c
                   for c in cmd]
        return _orig_run_command(cmd, *a, **kw)

    _bu.run_command = _run_command_ldw
    _bu._ldw_patched = True

N_CORES = 8
B, S, E = 32, 4096, 1024
H, D = 16, 64
B_LOC = B // N_CORES           # 4 batches per core
CH = 128                       # gathered rows per chunk (= SBUF partitions)
NE = E // 128                  # 8 e-blocks

F32 = mybir.dt.float32
F32R = mybir.dt.float32r
BF16 = mybir.dt.bfloat16
AF = mybir.ActivationFunctionType
NEG_BIAS = -30000.0            # additive pre-exp bias for padding rows


def build_nc(nch):
    """nch: chunks of 128 gathered rows per batch."""
    nc = bacc.Bacc("TRN2", target_bir_lowering=False, debug=False, num_devices=1)

    seq2 = nc.dram_tensor("seq2", [B_LOC * S, E], BF16, kind="ExternalInput").ap()
    s1t = nc.dram_tensor("s1t", [E, B_LOC], BF16, kind="ExternalInput").ap()
    idx = nc.dram_tensor("idx", [B_LOC * CH, nch], mybir.dt.int32, kind="ExternalInput").ap()
    bias = nc.dram_tensor("bias", [B_LOC * CH, nch], F32, kind="ExternalInput").ap()
    wq = nc.dram_tensor("wq", [E, E], BF16, kind="ExternalInput").ap()
    wkt = nc.dram_tensor("wkt", [E, E], BF16, kind="ExternalInput").ap()
    wv = nc.dram_tensor("wv", [E, E], BF16, kind="ExternalInput").ap()
    bq4 = nc.dram_tensor("bq4", [B_LOC, E], F32, kind="ExternalInput").ap()
    bv4 = nc.dram_tensor("bv4", [B_LOC, E], F32, kind="ExternalInput").ap()
    identb = nc.dram_tensor("identb", [128, 128], BF16, kind="ExternalInput").ap()
    onesd = nc.dram_tensor("onesd", [128, 1], BF16, kind="ExternalInput").ap()
    out = nc.dram_tensor("out", [B_LOC, E], F32, kind="ExternalOutput").ap()

    lin = os.environ.get("KLIN", "0") == "1"
    with tile.TileContext(nc, linearize=lin) as tc:
        _body(tc, nch, seq2, s1t, idx, bias, wq, wkt, wv, bq4, bv4, identb, onesd, out)
    nc.compile()
    return nc


def _body(tc, nch, seq2, s1t, idx, bias, wq, wkt, wv, bq4, bv4,
          identb, onesd, out):
    nc = tc.nc
    from contextlib import ExitStack

    with ExitStack() as stk:
        # ---- SBUF pools ------------------------------------------------
        consts = stk.enter_context(tc.tile_pool(name="consts", bufs=1))
        wpool = stk.enter_context(tc.tile_pool(name="wpool", bufs=1))
        chp = stk.enter_context(tc.tile_pool(name="chp", bufs=int(os.environ.get("KCHB", 24))))
        ctp = stk.enter_context(tc.tile_pool(name="ctp", bufs=int(os.environ.get("KCTB", 16))))
        wtp = stk.enter_context(tc.tile_pool(name="wtp", bufs=3))
        outp = stk.enter_context(tc.tile_pool(name="outp", bufs=1))

        # ---- constants -------------------------------------------------
        identb_sb = consts.tile([128, 128], BF16, tag="identb", name="identb")
        nc.sync.dma_start(identb_sb[:], identb[:])
        idx_sb = []
        bias_sb = []
        for b in range(B_LOC):
            t = consts.tile([CH, nch], mybir.dt.int32, tag=f"idx{b}", name=f"idx{b}")
            nc.gpsimd.dma_start(t[:], idx[b * CH:(b + 1) * CH, :])
            idx_sb.append(t)
            t = consts.tile([CH, nch], F32, tag=f"bias{b}", name=f"bias{b}")
            nc.sync.dma_start(t[:], bias[b * CH:(b + 1) * CH, :])
            bias_sb.append(t)
        ones_sb = consts.tile([128, 1], BF16, tag="ones", name="ones")
        nc.sync.dma_start(ones_sb[:], onesd[:])
        s1t_sb = []
        for j in range(NE):
            t = consts.tile([128, B_LOC], BF16, tag=f"s1t{j}", name=f"s1t{j}")
            nc.sync.dma_start(t[:], s1t[j * 128:(j + 1) * 128, :])
            s1t_sb.append(t)
        bq4_sb = consts.tile([B_LOC, E], F32, tag="bq4", name="bq4")
        nc.sync.dma_start(bq4_sb[:], bq4[:])
        bv4_sb = consts.tile([B_LOC, E], F32, tag="bv4", name="bv4")
        nc.sync.dma_start(bv4_sb[:], bv4[:])

        wq_sb, wkt_sb, wv_sb = [], [], []
        for j in range(NE):
            t = wpool.tile([128, E], BF16, tag=f"wq{j}", name=f"wq{j}")
            nc.sync.dma_start(t[:], wq[j * 128:(j + 1) * 128, :])
            wq_sb.append(t)
        for j in range(NE):
            t = wpool.tile([128, E], BF16, tag=f"wkt{j}", name=f"wkt{j}")
            nc.sync.dma_start(t[:], wkt[j * 128:(j + 1) * 128, :])
            wkt_sb.append(t)

        # ---- PSUM pools (8 banks total) --------------------------------
        tpp = stk.enter_context(tc.tile_pool(name="tpp", bufs=int(os.environ.get("KTPB", 3)), space="PSUM"))
        spp = stk.enter_context(tc.tile_pool(name="spp", bufs=2, space="PSUM"))
        ctxp = stk.enter_context(tc.tile_pool(name="ctxp", bufs=1, space="PSUM"))
        zp = stk.enter_context(tc.tile_pool(name="zp", bufs=1, space="PSUM"))

        z_ps = zp.tile([H, 512], F32, tag="z", name="z")

        def filler(n=3):
            # Dependency-free PE busywork into unused Z-bank columns: keeps
            # the HAM activity window hot so real matmuls stay at 2.4 GHz.
            # Safe: per-element has_written bits; cols 0:4 (real Z) untouched.
            for _ in range(n):
                nc.tensor.matmul(z_ps[:, 8:136], identb_sb[:, 0:H],
                                 identb_sb[:],
                                 start=False, stop=False,
                                 skip_group_check=True)

        # ---- chunk load helper (gather + transpose + evacuate) ---------
        cast_dma = os.environ.get("KCAST", "dma") == "dma"

        def load_chunk(b, c, ci):
            if cast_dma:
                ch = chp.tile([CH, E], BF16, tag="ch", name="ch")
                nc.gpsimd.indirect_dma_start(
                    out=ch[:], out_offset=None, in_=seq2[:],
                    in_offset=bass.IndirectOffsetOnAxis(
                        ap=idx_sb[b][:, c:c + 1], axis=0))
            else:
                chf = chp.tile([CH, E], F32, tag="chf", name="chf")
                nc.gpsimd.indirect_dma_start(
                    out=chf[:], out_offset=None, in_=seq2[:],
                    in_offset=bass.IndirectOffsetOnAxis(
                        ap=idx_sb[b][:, c:c + 1], axis=0))
                ch = chp.tile([CH, E], BF16, tag="ch", name="ch")
                if ci % 2 == 0:
                    nc.vector.tensor_copy(ch[:], chf[:])
                else:
                    nc.scalar.copy(ch[:], chf[:])
            # transpose the chunk: tp[:, j*128+s] = ch[s, j*128+e]
            tp = tpp.tile([CH, E], BF16, tag="tp", name="tp")
            for j in range(NE):
                nc.tensor.transpose(tp[:, j * 128:(j + 1) * 128],
                                    ch[:, j * 128:(j + 1) * 128],
                                    identb_sb[:])
            ct = ctp.tile([CH, E], BF16, tag="ct", name="ct")
            nc.vector.tensor_copy(ct[:], tp[:])
            return ch, ct

        # Warmup prepass: the PE executes in program order, so emit the first
        # chunks' transposes before the prologue matmuls - they only need the
        # gather + identity, which land long before the 6 MB of weights.
        filler(int(os.environ.get("KPREHEAT", 10)))
        warm = min(int(os.environ.get("KWARM", 4)), nch)
        pre = {}
        for c in range(warm):
            pre[(0, c)] = load_chunk(0, c, c)
        for j in range(NE):
            t = wpool.tile([128, E], BF16, tag=f"wv{j}", name=f"wv{j}")
            nc.sync.dma_start(t[:], wv[j * 128:(j + 1) * 128, :])
            wv_sb.append(t)

        # ================= prologue: q and qk =========================
        # q = seq1 @ Wq + bq   -> [B_LOC, E]
        q_ps = ctxp.tile([B_LOC, E], F32, tag="ctx", name="q_ps")
        for j in range(NE):
            for hf in range(2):
                nc.tensor.matmul(q_ps[:, hf * 512:(hf + 1) * 512], s1t_sb[j][:],
                                 wq_sb[j][:, hf * 512:(hf + 1) * 512],
                                 start=(j == 0), stop=(j == NE - 1),
                                 skip_group_check=True)
        q_sb = consts.tile([B_LOC, E], BF16, tag="q", name="q")
        nc.vector.tensor_add(q_sb[:], q_ps[:], bq4_sb[:])
        # qT blocks [128(hd), B_LOC] in bf16
        qt_sb = []
        for j in range(NE):
            ps = spp.tile([128, 64], BF16, tag="sp", name="qt_ps")
            nc.tensor.transpose(ps[:, 0:B_LOC], q_sb[:, j * 128:(j + 1) * 128],
                                identb_sb[0:B_LOC, 0:B_LOC])
            t = consts.tile([128, B_LOC], BF16, tag=f"qt{j}", name=f"qt{j}")
            nc.vector.tensor_copy(t[:], ps[:, 0:B_LOC])
            qt_sb.append(t)
        # qk[e, b*H+h] = sum_d WkT[h*64+d, e] * qT[h*64+d, b]
        # Block-diagonal trick: per hd-block, build qtbd [128, (b, par)] with
        # the other head-half zeroed, so one matmul against the FULL 128-row
        # WkT block computes both heads of the block at once (64 MMs not 128).
        qtbd = []
        for hj in range(NE):
            t = consts.tile([128, 2 * B_LOC], BF16, tag=f"qtbd{hj}",
                            name=f"qtbd{hj}")
            nc.vector.memset(t[:], 0.0)
            tr = t.rearrange("p (b q) -> p b q", q=2)
            nc.vector.tensor_copy(tr[0:64, :, 0:1], qt_sb[hj][0:64, :, None])
            nc.vector.tensor_copy(tr[64:128, :, 1:2], qt_sb[hj][64:128, :, None])
            qtbd.append(t)
        qk_sb = []
        for j in range(NE):
            ps = spp.tile([128, 64], F32, tag="sp", name="qk_ps")
            psr = ps.rearrange("p (b h) -> p b h", h=H)
            for hj in range(NE):
                nc.tensor.matmul(psr[:, :, 2 * hj:2 * hj + 2],
                                 wkt_sb[hj][:, j * 128:(j + 1) * 128],
                                 qtbd[hj][:],
                                 start=True, stop=True,
                                 skip_group_check=True)
            t = consts.tile([128, 64], BF16, tag=f"qk{j}", name=f"qk{j}")
            nc.scalar.copy(t[:], ps[:])
            qk_sb.append(t)

        if os.environ.get("KPART") == "qk":
            dbg = outp.tile([B_LOC, E], F32, tag="osb", name="osb")
            nc.vector.tensor_copy(dbg[:, 0:64], qk_sb[0][0:B_LOC, :])
            nc.sync.dma_start(out[:], dbg[:])
            return

        # ================= main loop ==================================
        zr_sb = consts.tile([H, B_LOC], F32, tag="zr", name="zr")
        ctxn = [consts.tile([H, E], BF16, tag=f"ctxn{b}", name=f"ctxn{b}")
                for b in range(B_LOC)]
        cxt_sb = [consts.tile([128, B_LOC * H], BF16, tag=f"cxt{j}", name=f"cxt{j}")
                  for j in range(NE)]
        n_b = int(os.environ.get("KNB", B_LOC))
        for b in range(n_b):
            ctx_ps = ctxp.tile([H, E], F32, tag="ctx", name="ctx")
            for c in range(nch):
                ci = b * nch + c
                if (b, c) in pre:
                    ch, ct = pre.pop((b, c))
                else:
                    ch, ct = load_chunk(b, c, ci)
                # score^T[s, h] accumulated over e-blocks
                sp = spp.tile([CH, H], F32, tag="sp", name="sp")
                for j in range(NE):
                    nc.tensor.matmul(sp[:], ct[:, j * 128:(j + 1) * 128],
                                     qk_sb[j][:, b * H:(b + 1) * H],
                                     start=(j == 0), stop=(j == NE - 1),
                                     skip_group_check=True)
                # attn weights: exp(score/8 + pad_bias); pad rows -> exactly 0
                wt = wtp.tile([CH, H], BF16, tag="wt", name="wt")
                nc.scalar.activation(wt[:], sp[:], AF.Exp,
                                     bias=bias_sb[b][:, c:c + 1],
                                     scale=1.0 / (D ** 0.5))
                # ctx[h, e] += wt^T @ ch ; Z[h, b] += wt^T @ ones
                last = (c == nch - 1)
                for hf in range(2):
                    nc.tensor.matmul(ctx_ps[:, hf * 512:(hf + 1) * 512], wt[:],
                                     ch[:, hf * 512:(hf + 1) * 512],
                                     start=(c == 0), stop=last,
                                     skip_group_check=True)
                nc.tensor.matmul(z_ps[:, b:b + 1], wt[:], ones_sb[:],
                                 start=(b == 0 and c == 0),
                                 stop=(b == n_b - 1 and c == nch - 1),
                                 skip_group_check=True)
            # normalize: ctxn[b] = ctx / Z   (bf16 out)
            nc.vector.reciprocal(zr_sb[:, b:b + 1], z_ps[:, b:b + 1])
            filler(14)
            nc.vector.tensor_scalar_mul(ctxn[b][:], ctx_ps[:], zr_sb[:, b:b + 1])
            # transpose ctxn into cxt blocks for the output projection
            for j in range(NE):
                ps = spp.tile([128, H], BF16, tag="sp", name="cxt_ps")
                nc.tensor.transpose(ps[:], ctxn[b][:, j * 128:(j + 1) * 128],
                                    identb_sb[0:H, 0:H])
                nc.scalar.copy(cxt_sb[j][:, b * H:(b + 1) * H], ps[:])

        if os.environ.get("KPART") == "ctx":
            dbg = outp.tile([B_LOC, E], F32, tag="osb", name="osb")
            nc.vector.tensor_copy(dbg[:], ctxn[0][0:B_LOC, :])
            nc.sync.dma_start(out[:], dbg[:])
            return

        # ================= finale: out = (ctx/Z) @ Wv + bv =============
        filler(20)
        out_sb = outp.tile([B_LOC, E], F32, tag="osb", name="osb")
        for h in range(H):
            op = spp.tile([B_LOC, 64], F32, tag="sp", name="op")
            for j in range(NE):
                lhs = cxt_sb[j].rearrange("p (b h) -> p h b", h=H)
                nc.tensor.matmul(op[:], lhs[:, h:h + 1, :],
                                 wv_sb[j][:, h * 64:(h + 1) * 64],
                                 start=(j == 0), stop=(j == NE - 1),
                                 skip_group_check=True)
            nc.vector.tensor_add(out_sb[:, h * 64:(h + 1) * 64], op[:],
                                 bv4_sb[:, h * 64:(h + 1) * 64])
        nc.sync.dma_start(out[:], out_sb[:])


# --------------------------------------------------------------------------
_NC_CACHE = {}

# test-harness knobs (the grading harness never touches these)
TRACE = False
TRACE_DIR = None
LAST_RESULTS = None


def _get_nc(nch):
    if nch not in _NC_CACHE:
        _NC_CACHE[nch] = build_nc(nch)
    return _NC_CACHE[nch]


def make_in_maps(inputs):
    """Host-side staging: mask -> gather indices + pad bias, weight reformat."""
    seq1 = np.asarray(inputs["seq1"], dtype=np.float32)   # [B,1,E]
    seq2 = np.asarray(inputs["seq2"], dtype=np.float32)   # [B,S,E]
    mask = np.asarray(inputs["mask"])                     # [B,1,1,S] int32
    Wq = np.asarray(inputs["Wq"], dtype=np.float32)
    # bk dropped: uniform per-row score shift, cancels exactly in softmax.
    Wk = np.asarray(inputs["Wk"], dtype=np.float32)
    Wv = np.asarray(inputs["Wv"], dtype=np.float32)
    bq = np.asarray(inputs["bq"], dtype=np.float32)
    bv = np.asarray(inputs["bv"], dtype=np.float32)

    mf = mask.reshape(B, S) != 0
    counts = mf.sum(1)
    nch = max(1, math.ceil(int(counts.max()) / CH))
    sp = nch * CH
    idx = np.zeros((B, sp), np.int32)
    bias = np.full((B, sp), NEG_BIAS, np.float32)
    for b in range(B):
        rows = np.nonzero(mf[b])[0]
        n = len(rows)
        bloc = b % B_LOC
        idx[b, :n] = bloc * S + rows
        idx[b, n:] = bloc * S
        bias[b, :n] = 0.0

    identb = np.eye(128, dtype=ml_dtypes.bfloat16)
    onesd = np.ones((128, 1), dtype=ml_dtypes.bfloat16)
    bq4 = np.tile(bq[None, :], (B_LOC, 1)).astype(np.float32)
    bv4 = np.tile(bv[None, :], (B_LOC, 1)).astype(np.float32)
    wkt = np.ascontiguousarray(Wk.T).astype(ml_dtypes.bfloat16)
    wvb = Wv.astype(ml_dtypes.bfloat16)

    in_maps = []
    for core in range(N_CORES):
        b0 = core * B_LOC
        # [B_LOC*CH, nch]: [b*128+p, c] = value for chunk c, partition p
        idxc = idx[b0:b0 + B_LOC].reshape(B_LOC, nch, CH).transpose(0, 2, 1)
        biasc = bias[b0:b0 + B_LOC].reshape(B_LOC, nch, CH).transpose(0, 2, 1)
        in_maps.append({
            "seq2": np.ascontiguousarray(seq2[b0:b0 + B_LOC].reshape(B_LOC * S, E)).astype(ml_dtypes.bfloat16),
            "s1t": np.ascontiguousarray(seq1[b0:b0 + B_LOC, 0, :].T).astype(ml_dtypes.bfloat16),
            "idx": np.ascontiguousarray(idxc.reshape(B_LOC * CH, nch)),
            "bias": np.ascontiguousarray(biasc.reshape(B_LOC * CH, nch)),
            "wq": Wq.astype(ml_dtypes.bfloat16), "wkt": wkt, "wv": wvb,
            "bq4": bq4, "bv4": bv4, "identb": identb, "onesd": onesd,
        })
    return nch, in_maps


def kernel(**inputs):
    nch, in_maps = make_in_maps(inputs)
    nc = _get_nc(nch)

    global LAST_RESULTS
    kwargs = {}
    if TRACE:
        kwargs = {"trace": True, "tmpdir": TRACE_DIR}
    # Retry: a previously-faulted NeuronCore can be left wedged
    # (NRT_EXEC_UNIT_UNRECOVERABLE) and recovers after reset/re-init.
    last_exc = None
    for attempt in range(4):
        try:
            res = run_bass_kernel_spmd(nc, in_maps, list(range(N_CORES)), **kwargs)
            break
        except Exception as e:  # noqa: BLE001
            last_exc = e
            time.sleep(10 * (attempt + 1))
    else:
        raise last_exc
    LAST_RESULTS = res
    out = np.concatenate([res.results[c]["out"] for c in range(N_CORES)], axis=0)
    return out.reshape(B, 1, E)


if __name__ == "__main__":
    t0 = time.time()
    nc = build_nc(17)
    print(f"build+compile(py): {time.time() - t0:.1f}s")
